# revision 44
# baseline (speedup 1.0000x reference)
"""Trainium2 Bass kernel for nn_DialogRater (RGCN message passing).

Contract: kernel(**inputs) takes the FULL unsharded inputs (as produced by
setup_inputs()) and returns the FULL output [256, 4] float32.

Strategy (8 NeuronCores, SPMD single program):
  - Graphs are partitioned contiguously across cores (32 graphs of 256
    nodes per core).  Within a core, the 8192 nodes are re-packed into 64
    blocks of 128 nodes by a greedy vector bin-pack so that every
    (block, relation) group has <= 128 incident edges; graph membership is
    recovered at pooling time with a one-hot, so the packing is free.
  - Per (block, relation): ONE 128-edge tile.  Edge features (x[src] rows,
    bf16) are gathered host-side into group-contiguous tiles and streamed
    with one DMA per block.  A scaled one-hot [(iota == dstcol) * (1/deg)]
    is built on the vector engine and the PE computes the transposed
    per-relation neighbor mean:
        U[k*128+f, dst] += G[:, k-chunk].T @ onehot     (out [128, 384])
    U is evicted to bf16 (alternating DVE/ACT) as B, which feeds the fused
    RGCN transform as the stationary operand:
        h[128 nodes, 384] = sum_{r,k} B_r[:, k-chunk].T @ W_rel[r][k-chunk]
                          + xT-tiles.T @ W_root (+ b_conv via K=1 matmul,
                            skipped when b_conv == 0)
    then relu on eviction (ACT) and graph-membership pooling matmuls
    (deferred one block to stay off the relu critical path) into a
    lifetime-long PSUM bank.
  - The tiny epilogue (mean /256, lin1, BatchNorm over the 256 graphs,
    head) runs on host in float64.
The per-input graph structure (node packing, tile schedule, one-hot
tables) is computed on the host and the Bass program is JIT-specialized
to it.
"""
import sys

sys.path.insert(0, "/opt/trn_rl_repo")

from contextlib import ExitStack

import numpy as np
import ml_dtypes

import concourse.bass as bass
import concourse.tile as tile
from concourse import bacc, mybir
from concourse.bass_utils import run_bass_kernel_spmd

NC = 8
N_NODES = 65536
D = 384
H = 384
N_REL = 9
GRAPH = 256                     # nodes per graph
NODES_PER_CORE = N_NODES // NC  # 8192
NB = NODES_PER_CORE // 128      # 64 blocks of 128 nodes per core
NG = 32                         # graphs per core
TILE_E = 128                    # edges per tile
P = 128
BN_EPS = 1e-5

bf16 = ml_dtypes.bfloat16


def _pack_nodes(deg):
    """Greedy vector bin-pack: 8192 nodes -> 64 blocks of 128 nodes with
    per-relation edge loads as even as possible.  deg: [8192, 9] int."""
    n = deg.shape[0]
    order = np.argsort(-deg.sum(axis=1), kind="stable")
    loads = np.zeros((NB, N_REL), np.int64)
    fill = np.zeros(NB, np.int64)
    blk_of = np.zeros(n, np.int64)
    slot_of = np.zeros(n, np.int64)
    for nd in order:
        sc = np.max(loads + deg[nd][None, :], axis=1).astype(np.float64)
        sc[fill >= P] = np.inf
        b = int(np.argmin(sc))
        blk_of[nd] = b
        slot_of[nd] = fill[b]
        fill[b] += 1
        loads[b] += deg[nd]
    return blk_of, slot_of


def _preprocess(x, src, dst, et):
    cnt = np.bincount(dst * N_REL + et, minlength=N_NODES * N_REL).reshape(
        N_NODES, N_REL)
    rc_full = (1.0 / np.maximum(cnt, 1.0)).astype(np.float32)

    graph_of_node = np.arange(N_NODES) // GRAPH
    core_of_node = np.arange(N_NODES) // NODES_PER_CORE

    per_core_pack = []
    counts = np.zeros((NC, NB, N_REL), np.int64)
    for c in range(NC):
        nodes = np.arange(c * NODES_PER_CORE, (c + 1) * NODES_PER_CORE)
        deg = cnt[nodes]
        blk_of, slot_of = _pack_nodes(deg)
        per_core_pack.append((blk_of, slot_of))
        ecore = core_of_node[dst] == c
        b_of_e = blk_of[dst[ecore] - c * NODES_PER_CORE]
        r_of_e = et[ecore]
        counts[c] = np.bincount(b_of_e * N_REL + r_of_e,
                                minlength=NB * N_REL).reshape(NB, N_REL)

    ntiles = np.ceil(counts / TILE_E).astype(np.int64).max(axis=0)
    schedule = []           # list of (block, rel, ntiles)
    for b in range(NB):
        for r in range(N_REL):
            nt = int(ntiles[b, r])
            if nt > 0:
                schedule.append((b, r, nt))
    T_flat = sum(nt for _, _, nt in schedule)
    goff = {}
    off = 0
    for b, r, nt in schedule:
        goff[(b, r)] = off
        off += nt

    per_core = []
    for c in range(NC):
        blk_of, slot_of = per_core_pack[c]
        srcidx = np.zeros((T_flat, TILE_E), np.int64)
        dcol = np.full((T_flat, TILE_E), -1.0, np.float32)
        rcv = np.zeros((T_flat, TILE_E), np.float32)
        ecore = np.nonzero(core_of_node[dst] == c)[0]
        dloc = dst[ecore] - c * NODES_PER_CORE
        b_of_e = blk_of[dloc]
        s_of_e = slot_of[dloc]
        r_of_e = et[ecore]
        gid = b_of_e * N_REL + r_of_e
        order = np.argsort(gid, kind="stable")
        gid_s = gid[order]
        starts = np.zeros(NB * N_REL + 1, np.int64)
        starts[1:] = np.cumsum(np.bincount(gid_s, minlength=NB * N_REL))
        for b in range(NB):
            for r in range(N_REL):
                if (b, r) not in goff:
                    continue
                sel = order[starts[b * N_REL + r]:starts[b * N_REL + r + 1]]
                k = len(sel)
                if k:
                    t0 = goff[(b, r)]
                    flat = np.arange(k)
                    e = ecore[sel]
                    srcidx[t0 + flat // TILE_E, flat % TILE_E] = src[e]
                    dcol[t0 + flat // TILE_E, flat % TILE_E] = s_of_e[sel]
                    rcv[t0 + flat // TILE_E, flat % TILE_E] = \
                        rc_full[dst[e], r]
        # node table per block: global node id at (block, slot)
        nodes = np.arange(c * NODES_PER_CORE, (c + 1) * NODES_PER_CORE)
        node_at = np.zeros((NB, P), np.int64)
        node_at[blk_of, slot_of] = nodes
        gcol = (graph_of_node[node_at] - c * NG).astype(np.float32)  # [NB,P]
        per_core.append(dict(
            srcidx_T=np.ascontiguousarray(srcidx.T),
            dstcol_T=np.ascontiguousarray(dcol.T),
            rc_T=np.ascontiguousarray(rcv.T),
            node_at=node_at,
            gcol_T=np.ascontiguousarray(gcol.T),    # [P, NB]
        ))
    return schedule, T_flat, goff, per_core


def _make_xg(xb, srcidx_T, schedule, goff):
    """Per-block p-major gathered edge features, concatenated: for each
    block, rows [128, ntb] of src indices -> xb rows."""
    parts = []
    by_block = {}
    for (b, r, nt) in schedule:
        by_block.setdefault(b, []).append((r, nt))
    for b in range(NB):
        t0 = goff[(b, by_block[b][0][0])]
        ntb = sum(nt for _, nt in by_block[b])
        idx = srcidx_T[:, t0:t0 + ntb].reshape(-1)
        parts.append(xb[idx])
    return np.ascontiguousarray(np.concatenate(parts, axis=0))


def _make_weights(W_rel, W_root):
    tiles = []
    for r in range(N_REL):
        for k in range(3):
            tiles.append(W_rel[r, k * P:(k + 1) * P, :])
    for k in range(3):
        tiles.append(W_root[k * P:(k + 1) * P, :])
    w = np.stack(tiles).astype(bf16)              # [30, 128, 384]
    # pre-transpose to [128, 30*384] so the DMA is contiguous per partition
    return np.ascontiguousarray(w.transpose(1, 0, 2).reshape(P, 30 * H))


def _make_xt_tiles(x, node_at):
    """xt[b] = [128 f-chunk, 128 nodes] x 3 chunks, per block, in packed
    node order: [NB, 128, 3*128] laid as [NB*128 rows, 3*128]."""
    out = np.zeros((NB, P, 3 * P), np.float32)
    for b in range(NB):
        xb = x[node_at[b]]                     # [128 nodes, 384]
        for k in range(3):
            out[b, :, k * P:(k + 1) * P] = xb[:, k * P:(k + 1) * P].T
    return np.ascontiguousarray(out.reshape(NB * P, 3 * P).astype(bf16))


def _build(schedule, T_flat, with_bias):
    nc = bacc.Bacc("TRN2", target_bir_lowering=False, debug=False,
                   enable_asserts=False, num_devices=NC)
    bfd = mybir.dt.bfloat16
    f32 = mybir.dt.float32

    xg_d = nc.dram_tensor("xg", [T_flat * P, D], bfd,
                          kind="ExternalInput").ap()
    dstcol_d = nc.dram_tensor("dstcol", [P, T_flat], f32,
                              kind="ExternalInput").ap()
    rc_d = nc.dram_tensor("rc", [P, T_flat], f32, kind="ExternalInput").ap()
    gcol_d = nc.dram_tensor("gcol", [P, NB], f32, kind="ExternalInput").ap()
    wstack_d = nc.dram_tensor("wstack", [P, 30 * H], bfd,
                              kind="ExternalInput").ap()
    xt_d = nc.dram_tensor("xt", [NB * P, 3 * P], bfd,
                          kind="ExternalInput").ap()
    bconv_d = nc.dram_tensor("bconv", [1, H], bfd, kind="ExternalInput").ap()
    iota_d = nc.dram_tensor("iota", [P, P], bfd, kind="ExternalInput").ap()
    iota32_d = nc.dram_tensor("iota32", [P, NG], bfd,
                              kind="ExternalInput").ap()
    pool_out_d = nc.dram_tensor("pool_out", [NG, H], f32,
                                kind="ExternalOutput").ap()

    by_block = [[] for _ in range(NB)]
    gt = 0
    for (b, r, nt) in schedule:
        by_block[b].append((r, nt, gt))
        gt += nt
    blk_off = []
    off = 0
    for b in range(NB):
        blk_off.append(off)
        off += sum(nt for _, nt, _ in by_block[b])

    with tile.TileContext(nc) as tc, ExitStack() as ctx:
        const = ctx.enter_context(tc.tile_pool(name="const", bufs=1))
        gpool = ctx.enter_context(tc.tile_pool(name="gpool", bufs=6))
        ohpool = ctx.enter_context(tc.tile_pool(name="ohpool", bufs=16))
        btpool = ctx.enter_context(tc.tile_pool(name="btpool", bufs=46))
        xtpool = ctx.enter_context(tc.tile_pool(name="xtpool", bufs=6))
        hsbpool = ctx.enter_context(tc.tile_pool(name="hsbpool", bufs=3))
        mpool = ctx.enter_context(tc.tile_pool(name="mpool", bufs=3))
        utps = ctx.enter_context(tc.tile_pool(name="utps", bufs=3,
                                              space="PSUM"))
        warmps = ctx.enter_context(tc.tile_pool(name="warmps", bufs=1,
                                                space="PSUM"))
        hps = ctx.enter_context(tc.tile_pool(name="hps", bufs=3,
                                             space="PSUM"))
        poolps = ctx.enter_context(tc.tile_pool(name="poolps", bufs=1,
                                                space="PSUM"))

        iota_sb = const.tile([P, P], bfd, tag="iota")
        nc.sync.dma_start(iota_sb[:], iota_d[:])
        iota32_sb = const.tile([P, NG], bfd, tag="iota32")
        nc.sync.dma_start(iota32_sb[:], iota32_d[:])
        # weights in 3 pieces staged between early edge-tile DMAs so
        # transform(0) can start before the full 3 MB lands
        w_parts = [const.tile([P, 10 * H], bfd, name=f"w{j}", tag=f"w{j}")
                   for j in range(3)]

        def w_at(idx):          # idx in [0, 30): weight tile index
            return w_parts[idx // 10][:, (idx % 10) * H:(idx % 10 + 1) * H]

        # split the per-tile tables so the first blocks' one-hot builds do
        # not wait for the full-table DMA (separate tiles => separate deps)
        T0 = min(T_flat, 96)
        dstcol_a = const.tile([P, T0], f32, tag="dstcol_a")
        nc.sync.dma_start(dstcol_a[:], dstcol_d[:, :T0])
        rc_a = const.tile([P, T0], f32, tag="rc_a")
        nc.sync.dma_start(rc_a[:], rc_d[:, :T0])
        dstcol_b = const.tile([P, T_flat - T0], f32, tag="dstcol_b")
        rc_b = const.tile([P, T_flat - T0], f32, tag="rc_b")

        def dstcol_at(col):
            return (dstcol_a[:, col:col + 1] if col < T0
                    else dstcol_b[:, col - T0:col - T0 + 1])

        def rc_at(col):
            return (rc_a[:, col:col + 1] if col < T0
                    else rc_b[:, col - T0:col - T0 + 1])

        gcol_sb = const.tile([P, NB], f32, tag="gcol")
        nc.sync.dma_start(gcol_sb[:], gcol_d[:])
        bconv_sb = const.tile([1, H], bfd, tag="bconv")
        nc.sync.dma_start(bconv_sb[:], bconv_d[:])
        ones_row = const.tile([1, P], bfd, tag="ones_row")
        nc.vector.memset(ones_row[:], 1.0)

        # keep the PE busy (and its clock ramping) while the first input
        # DMAs land: dummy matmuls with no DMA dependencies
        warm = warmps.tile([P, P], f32, tag="warm")
        for _ in range(36):
            nc.tensor.matmul(out=warm[:], lhsT=ones_row[:], rhs=ones_row[:],
                             start=True, stop=True)

        pool_tile = poolps.tile([NG, H], f32, tag="pool")
        pending_pool = []

        def flush_pool(last=False):
            # membership one-hot as the stationary: one matmul per block,
            # out [32 graphs, 384].
            for (pb, h_sb, memb) in pending_pool:
                nc.tensor.matmul(
                    out=pool_tile[:], lhsT=memb[:], rhs=h_sb[:],
                    start=(pb == 0), stop=(last and pb == NB - 1),
                )
            pending_pool.clear()

        DEFER = 3   # transform lags scatter by this many blocks so the
        #             3 MB weight DMA hides under early scatter work
        ev_cnt = 0
        bts_of = {}
        for step in range(NB + DEFER):
            if step < NB:
                b = step
                groups = by_block[b]
                ntb = sum(nt for _, nt, _ in groups)
                G = gpool.tile([P, ntb * D], bfd, tag="g")
                nc.sync.dma_start(
                    G[:].rearrange("p (n d) -> p n d", n=ntb),
                    xg_d[blk_off[b] * P:(blk_off[b] + ntb) * P, :].rearrange(
                        "(p n) d -> p n d", p=P))
                if step in (1, 2, 3):
                    j = step - 1
                    nc.sync.dma_start(
                        w_parts[j][:], wstack_d[:, j * 10 * H:(j + 1) * 10 * H])
                if step == 8:
                    nc.sync.dma_start(dstcol_b[:], dstcol_d[:, T0:])
                if step == 9:
                    nc.sync.dma_start(rc_b[:], rc_d[:, T0:])

                bts = {}
                for (r, nt, g0) in groups:
                    t_loc0 = g0 - blk_off[b]
                    ut = utps.tile([P, 3 * P], f32, tag="ut")
                    for t in range(nt):
                        col = g0 + t
                        oh = ohpool.tile([P, P], bfd, tag="oh")
                        nc.vector.tensor_scalar(
                            out=oh[:], in0=iota_sb[:],
                            scalar1=dstcol_at(col),
                            scalar2=rc_at(col),
                            op0=mybir.AluOpType.is_equal,
                            op1=mybir.AluOpType.mult,
                        )
                        for k in range(3):
                            # single start per ut bank: start=True clears the
                            # has_written bits of the WHOLE bank
                            nc.tensor.matmul(
                                out=ut[:, k * P:(k + 1) * P],
                                lhsT=G[:, (t_loc0 + t) * D + k * P:
                                       (t_loc0 + t) * D + (k + 1) * P],
                                rhs=oh[:], start=(t == 0 and k == 0),
                                stop=(t == nt - 1 and k == 2),
                            )
                    bt = btpool.tile([P, 3 * P], bfd, tag="bt")
                    if ev_cnt % 2 == 0:
                        nc.vector.tensor_copy(bt[:], ut[:])
                    else:
                        nc.scalar.copy(bt[:], ut[:])
                    ev_cnt += 1
                    bts[r] = bt
                bts_of[b] = bts

            if step < DEFER:
                continue
            tb = step - DEFER
            groups = by_block[tb]
            bts = bts_of.pop(tb)
            xt = xtpool.tile([P, 3 * P], bfd, tag="xt")
            nc.sync.dma_start(xt[:], xt_d[tb * P:(tb + 1) * P, :])
            flush_pool()

            h_ps = hps.tile([P, H], f32, tag="h")
            mms = [(bts[r][:, k * P:(k + 1) * P], w_at(r * 3 + k))
                   for (r, nt, g0) in groups for k in range(3)]
            for k in range(3):
                mms.append((xt[:, k * P:(k + 1) * P], w_at(27 + k)))
            for i, (lhsT, rhs) in enumerate(mms):
                nc.tensor.matmul(out=h_ps[:], lhsT=lhsT, rhs=rhs,
                                 start=(i == 0),
                                 stop=(not with_bias and i == len(mms) - 1))
            if with_bias:
                nc.tensor.matmul(out=h_ps[:], lhsT=ones_row[:],
                                 rhs=bconv_sb[:], start=False, stop=True)
            h_sb = hsbpool.tile([P, H], bfd, tag="hsb")
            nc.scalar.activation(out=h_sb[:], in_=h_ps[:],
                                 func=mybir.ActivationFunctionType.Relu)
            memb = mpool.tile([P, NG], bfd, tag="memb")
            nc.vector.tensor_scalar(
                out=memb[:], in0=iota32_sb[:],
                scalar1=gcol_sb[:, tb:tb + 1], scalar2=None,
                op0=mybir.AluOpType.is_equal,
            )
            pending_pool.append((tb, h_sb, memb))
            if tb == NB - 1:
                flush_pool(last=True)

        pool_ev = const.tile([NG, H], f32, tag="poolev")
        nc.scalar.copy(pool_ev[:], pool_tile[:])
        nc.sync.dma_start(pool_out_d[:], pool_ev[:])

    nc.compile()
    return nc


def kernel(x, edge_index, edge_type, batch_size,
           W_rel, W_root, b_conv, W_lin1, b_lin1,
           bn_gamma, bn_beta, W_head, b_head):
    x = np.asarray(x, np.float32)
    edge_index = np.asarray(edge_index)
    edge_type = np.asarray(edge_type)
    batch_size = int(batch_size)
    W_rel = np.asarray(W_rel, np.float32)
    W_root = np.asarray(W_root, np.float32)
    b_conv = np.asarray(b_conv, np.float32)

    src = edge_index[0].astype(np.int64)
    dst = edge_index[1].astype(np.int64)
    et = edge_type.astype(np.int64)

    schedule, T_flat, goff, per_core = _preprocess(x, src, dst, et)
    nc = _build(schedule, T_flat, with_bias=bool(np.any(b_conv)))

    xb = np.ascontiguousarray(x.astype(bf16))
    Wstack = _make_weights(W_rel, W_root)
    bconv = np.ascontiguousarray(b_conv.astype(bf16)[None, :])
    iota = np.ascontiguousarray(
        np.broadcast_to(np.arange(P, dtype=np.float32), (P, P)).astype(bf16))
    iota32 = np.ascontiguousarray(
        np.broadcast_to(np.arange(NG, dtype=np.float32),
                        (P, NG)).astype(bf16))

    in_maps = []
    for c in range(NC):
        pc = per_core[c]
        in_maps.append({
            "xg": _make_xg(xb, pc["srcidx_T"], schedule, goff),
            "dstcol": pc["dstcol_T"],
            "rc": pc["rc_T"],
            "gcol": pc["gcol_T"],
            "wstack": Wstack,
            "xt": _make_xt_tiles(x, pc["node_at"]),
            "bconv": bconv,
            "iota": iota,
            "iota32": iota32,
        })

    res = run_bass_kernel_spmd(nc, in_maps, core_ids=list(range(NC)))

    # host epilogue: mean-pool scale, lin1, BatchNorm (batch stats), head
    pooled = np.zeros((batch_size, H), np.float64)
    for c in range(NC):
        po = np.asarray(res.results[c]["pool_out"], np.float64)  # [32, 384]
        pooled[c * NG:(c + 1) * NG, :] = po / GRAPH
    g = pooled @ np.asarray(W_lin1, np.float64) + np.asarray(b_lin1,
                                                            np.float64)
    mu = g.mean(axis=0)
    var = g.var(axis=0)
    g = (g - mu) / np.sqrt(var + BN_EPS) * np.asarray(bn_gamma, np.float64) \
        + np.asarray(bn_beta, np.float64)
    out = g @ np.asarray(W_head, np.float64) + np.asarray(b_head, np.float64)
    return np.squeeze(out.astype(np.float32))
